# revision 22
# baseline (speedup 1.0000x reference)
"""3-layer dense GAT (N=4096, F=512, H=8 heads, D=64) on 8 TRN2 NeuronCores.

Strategy (1D row-parallel, exp(f2)-folded attention):
  - Each core owns LOCAL=512 query rows i. Per layer, each core computes its
    local hext = x_local @ [W | W@a1 | W@a2], so f1/f2 come out as extra
    matmul columns.
  - Key identity: with ec_j = exp(f2_j), r_j = exp((a-1) f2_j),
      max(exp(f1_i)exp(f2_j), exp(a f1_i)exp(a f2_j))
        = ec_j * max(exp(f1_i), exp(a f1_i) * r_j).
    So ec is folded into the stationary matrix (ec*h, and ec replaces the
    ones column so the softmax denominator Z still falls out of the same
    matmul), and the per-tile work drops to ONE scalar_tensor_tensor
    (p = max(EA, EB*r), per-partition scalar r) + ONE tensor_tensor mask
    multiply (merged over j-chunk pairs, split across Vector and GpSimd).
  - The scaling ec*h runs on the otherwise-idle Scalar engine as
    activation-Copy with a per-partition scale during PSUM extraction.
  - AllGather is issued per head (8 smaller collectives) so head 0's
    attention overlaps the remaining gathers and staging DMAs.
  - Matmul: out[d,i] += GS_h[j, 0:65].T @ pm[j,i] accumulated over 32
    j-chunks; column 64 of GS_h is ec, so PSUM row 64 is Z. h' = U/Z, ELU;
    the [d,i] orientation is the next layer's lhsT, so no transposes.
"""

import numpy as np
import ml_dtypes

import concourse.bass as bass
import concourse.mybir as mybir
from concourse import bacc, tile, masks
from concourse.bass_utils import run_bass_kernel_spmd

N = 4096
F = 512
D = 64
H = 8
NCORES = 8
LOCAL = N // NCORES          # 512 query rows per core
JC = N // 128                # 32 j-chunks
IC = LOCAL // 128            # 4 local i-chunks
FC = F // 128                # 4 contraction chunks
NL = 3
SLOT = 66                    # per-head cols: 64 ec*h + ec + r
CH = H * SLOT                # 528: per-j-chunk stride in GS
ALPHA = 0.2
f32 = mybir.dt.float32
bf16 = mybir.dt.bfloat16
BF = ml_dtypes.bfloat16
OP = mybir.AluOpType
AF = mybir.ActivationFunctionType


def build_nc():
    nc = bacc.Bacc(None, target_bir_lowering=False, num_devices=NCORES)

    xT_d = nc.dram_tensor("xT", [F, LOCAL], bf16, kind="ExternalInput")
    maskT_d = nc.dram_tensor("maskT", [N, LOCAL], bf16, kind="ExternalInput")
    wext_d = nc.dram_tensor("wext", [NL, F, H * SLOT], bf16, kind="ExternalInput")
    wfa_d = nc.dram_tensor("wfa", [NL, F, 2 * H], bf16, kind="ExternalInput")
    out_d = nc.dram_tensor("outT", [D, LOCAL], f32, kind="ExternalOutput")

    with tile.TileContext(nc) as tc:
        with (
            tc.tile_pool(name="persist", bufs=1) as pp,
            tc.tile_pool(name="ident", bufs=1) as ident_pool,
            tc.tile_pool(name="hc", bufs=4) as hc_pool,
            tc.tile_pool(name="ecd", bufs=3) as ecd_pool,
            tc.tile_pool(name="ea8", bufs=2) as ea8_pool,
            tc.tile_pool(name="ptile", bufs=6) as p_pool,
            tc.tile_pool(name="pmtile", bufs=6) as pm_pool,
            tc.tile_pool(name="norm", bufs=2) as nm_pool,
            tc.tile_pool(name="tb", bufs=6) as tb_pool,
            tc.tile_pool(name="psA", bufs=2, space="PSUM") as psA,
            tc.tile_pool(name="psB", bufs=3, space="PSUM") as psB,
            tc.tile_pool(name="psT", bufs=1, space="PSUM") as psT,
            tc.tile_pool(name="dram", bufs=1, space="DRAM") as dram,
        ):
            # ---- persistent SBUF ----
            XT = pp.tile([128, FC * LOCAL], bf16, tag="XT")        # x^T local
            MASK = pp.tile([128, JC * LOCAL], bf16, tag="MASK")    # mask^T
            WEXT = pp.tile([128, NL * FC * H * SLOT], bf16, tag="WEXT")
            GS = pp.tile([128, JC * CH], bf16, tag="GS")           # gathered stationary
            EAB = pp.tile([128, H * 2 * LOCAL], bf16, tag="EAB")   # exp(f1) bcasts
            WFA = pp.tile([128, NL * FC * 2 * H], bf16, tag="WFA")
            RF = pp.tile([128, H * JC], f32, tag="RF")             # f32 r cols (h-major)
            F1B = pp.tile([128, H * LOCAL], bf16, tag="F1B")       # raw f1 bcasts
            F2G = pp.tile([128, JC * H], f32, tag="F2G")           # gathered f2 (jc-major)
            F2GA = pp.tile([128, JC * H], f32, tag="F2GA")         # alpha * f2
            ACCa = pp.tile([D, LOCAL], f32, tag="ACCa")            # layer-3 head mean
            ACCb = pp.tile([D, LOCAL], f32, tag="ACCb")
            OUTS = pp.tile([D, LOCAL], f32, tag="OUTS")
            IDENT = ident_pool.tile([128, 128], f32)

            # DRAM bounce buffers: per-head-PAIR local slabs + gathered slabs
            LGPs = [
                dram.tile(
                    [LOCAL, 2 * SLOT], bf16, tag=f"LGP{p}", name=f"LGP{p}"
                )
                for p in range(H // 2)
            ]
            GGPs = [
                [
                    dram.tile(
                        [N, 2 * SLOT], bf16, tag=f"GGP{l}_{p}",
                        addr_space="Shared", name=f"GGP{l}_{p}",
                    )
                    for p in range(H // 2)
                ]
                for l in range(NL)
            ]
            EDR = dram.tile([3, 16, LOCAL], bf16, tag="EDR")  # f1 bounce
            EDR4 = dram.tile([16, 2 * LOCAL], bf16, tag="EDR4")  # [ea|eb] rows
            LG2s = [
                dram.tile([LOCAL, 4], f32, tag=f"LG2{g}", name=f"LG2{g}")
                for g in range(2)
            ]
            GG2s = [
                [
                    dram.tile(
                        [N, 4], f32, tag=f"GG2_{l}_{g}", addr_space="Shared",
                        name=f"GG2_{l}_{g}",
                    )
                    for g in range(2)
                ]
                for l in range(NL)
            ]

            # views
            X2 = XT[:].rearrange("p (fc i) -> p fc i", i=LOCAL)
            M2 = MASK[:].rearrange("p (c i) -> p c i", i=LOCAL)
            W4 = WEXT[:].rearrange("p (l fc s) -> p l fc s", l=NL, fc=FC)
            WFA4 = WFA[:].rearrange("p (l fc s) -> p l fc s", l=NL, fc=FC)
            GS2 = GS[:].rearrange("p (c s) -> p c s", s=CH)
            RF2 = RF[:].rearrange("p (h c) -> p h c", h=H)
            F2Gv = F2G[:].rearrange("p (c s) -> p c s", s=H)
            F2GAv = F2GA[:].rearrange("p (c s) -> p c s", s=H)
            EA4 = EAB[:].rearrange("p (h t i) -> p h t i", h=H, t=2)

            # ---- one-time loads ----
            nc.sync.dma_start(X2, xT_d[:].rearrange("(fc p) i -> p fc i", p=128))
            nc.sync.dma_start(M2, maskT_d[:].rearrange("(c p) i -> p c i", p=128))
            nc.sync.dma_start(
                W4, wext_d[:].rearrange("l (fc p) s -> p l fc s", p=128)
            )
            nc.sync.dma_start(
                WFA4, wfa_d[:].rearrange("l (fc p) s -> p l fc s", p=128)
            )
            masks.make_identity(nc, IDENT[:])
            NEG1 = pp.tile([128, 1], f32, tag="NEG1")
            nc.gpsimd.memset(NEG1[:], -1.0)


            def emit_tail(l, h, pb):
                # normalize by Z (PSUM row 64) and apply ELU
                # 1/Z: DMA-reshape Z [1,512] -> [128,4] so the iterative
                # divide runs 4-deep across 128 lanes, then reshape back.
                zrow1 = nm_pool.tile([1, LOCAL], f32, tag="zrow1")
                nc.scalar.copy(zrow1[:], pb[D : D + 1, :])
                zz4 = nm_pool.tile([128, LOCAL // 128], f32, tag="zz4")
                nc.sync.dma_start(zz4[:], zrow1[:])
                rz4 = nm_pool.tile([128, LOCAL // 128], f32, tag="rz4")
                nc.vector.reciprocal(rz4[:], zz4[:])
                r1 = nm_pool.tile([1, LOCAL], f32, tag="r1")
                nc.sync.dma_start(r1[:], rz4[:])
                rb2 = nm_pool.tile([D, LOCAL], f32, tag="rb2")
                nc.gpsimd.partition_broadcast(rb2[:], r1[:], channels=D)
                y = nm_pool.tile([D, LOCAL], f32, tag="y")
                nc.vector.tensor_tensor(y[:], pb[0:D, :], rb2[:], OP.mult)
                ee = nm_pool.tile([D, LOCAL], f32, tag="ee")
                nc.scalar.activation(ee[:], y[:], AF.Exp)
                ry = nm_pool.tile([D, LOCAL], f32, tag="ry")
                nc.scalar.activation(ry[:], y[:], AF.Relu)
                z1 = nm_pool.tile([D, LOCAL], f32, tag="z1")
                nc.vector.scalar_tensor_tensor(
                    z1[:], ee[:], 1.0, ry[:], OP.min, OP.add
                )
                if l < NL - 1:
                    poff = (h % 2) * D
                    dst = X2[poff : poff + D, h // 2, :]
                    nc.scalar.activation(dst, z1[:], AF.Identity, bias=NEG1[0:D, :])
                else:
                    ey = nm_pool.tile([D, LOCAL], f32, tag="ey")
                    nc.scalar.activation(ey[:], z1[:], AF.Identity, bias=NEG1[0:D, :])
                    if h == 0:
                        nc.vector.tensor_copy(ACCa[:], ey[:])
                    else:
                        acc_src, dst_acc = (
                            (ACCa, ACCb) if h % 2 == 1 else (ACCb, ACCa)
                        )
                        nc.vector.tensor_tensor(
                            dst_acc[:], acc_src[:], ey[:], OP.add
                        )

            pending_tails = []

            for l in range(NL):
                while pending_tails:
                    emit_tail(*pending_tails.pop(0))
                # ---- Phase A2 (first): f1/f2 rows via matmul, exp, bcast ----
                psf = psT.tile([2 * H, LOCAL], f32, tag="psf")
                for fc in range(FC):
                    nc.tensor.matmul(
                        psf[:],
                        WFA4[:, l, fc, :],
                        X2[:, fc, :],
                        start=(fc == 0),
                        stop=(fc == FC - 1),
                    )
                ea8 = ea8_pool.tile([16, LOCAL], bf16, tag="ea8")
                eb8 = ea8_pool.tile([16, LOCAL], bf16, tag="eb8")
                nc.scalar.activation(ea8[:], psf[:], AF.Exp)
                nc.scalar.activation(eb8[:], psf[:], AF.Exp, scale=ALPHA)
                fr16 = ea8_pool.tile([16, LOCAL], bf16, tag="fr16")
                nc.vector.tensor_copy(fr16[:], psf[:])

                # ---- Phase A: local hext = x_local @ Wext, 4 heads/group ----
                for g in range(2):
                    for ic in range(IC):
                        ps = psA.tile([128, 4 * SLOT], f32, tag="psA")
                        for fc in range(FC):
                            nc.tensor.matmul(
                                ps[:],
                                X2[:, fc, ic * 128 : (ic + 1) * 128],
                                W4[:, l, fc, g * 4 * SLOT : (g + 1) * 4 * SLOT],
                                start=(fc == 0),
                                stop=(fc == FC - 1),
                            )
                        psv = ps[:].rearrange("p (h s) -> p h s", s=SLOT)
                        rows = slice(ic * 128, (ic + 1) * 128)
                        sg4 = hc_pool.tile([128, 4 * SLOT], bf16, tag="sg4")
                        sg4v = sg4[:].rearrange("p (h s) -> p h s", s=SLOT)
                        # folded rows: [ec*h | ec | r]; f2 staged (f32)
                        # separately as the activation-branch bias
                        ecf = ecd_pool.tile([128, 4], f32, tag="ecf")
                        nc.scalar.activation(ecf[:], psv[:, :, D + 1], AF.Exp)
                        rf = ecd_pool.tile([128, 4], f32, tag="rf")
                        nc.scalar.activation(
                            rf[:], psv[:, :, D + 1], AF.Exp, scale=ALPHA - 1.0
                        )
                        for hh in range(4):
                            nc.scalar.activation(
                                sg4v[:, hh, 0:D],
                                psv[:, hh, 0:D],
                                AF.Copy,
                                scale=ecf[:, hh : hh + 1],
                            )
                        nc.vector.tensor_copy(sg4v[:, :, D], ecf[:])
                        nc.vector.tensor_copy(sg4v[:, :, D + 1], rf[:])
                        f2c = ecd_pool.tile([128, 4], f32, tag="f2c")
                        nc.scalar.copy(f2c[:], psv[:, :, D + 1])
                        nc.sync.dma_start(LG2s[g][rows, :], f2c[:])
                        for hh in range(4):
                            h = g * 4 + hh
                            nc.sync.dma_start(
                                LGPs[h // 2][
                                    rows, (h % 2) * SLOT : (h % 2 + 1) * SLOT
                                ],
                                sg4v[:, hh, :],
                            )
                    # this head-group's slabs are complete: gather them now
                    def _agp(p):
                        nc.gpsimd.collective_compute(
                            "AllGather",
                            OP.bypass,
                            replica_groups=[list(range(NCORES))],
                            ins=[LGPs[p].opt()],
                            outs=[GGPs[l][p].opt()],
                        )
                    _agp(2 * g)
                    nc.gpsimd.collective_compute(
                        "AllGather",
                        OP.bypass,
                        replica_groups=[list(range(NCORES))],
                        ins=[LG2s[g].opt()],
                        outs=[GG2s[l][g].opt()],
                    )
                    _agp(2 * g + 1)
                    nc.sync.dma_start(
                        F2Gv[:, :, g * 4 : (g + 1) * 4],
                        GG2s[l][g][:].rearrange("(c p) s -> p c s", p=128),
                    )
                    nc.vector.tensor_scalar(
                        F2GAv[:, :, g * 4 : (g + 1) * 4],
                        F2Gv[:, :, g * 4 : (g + 1) * 4],
                        ALPHA - 1.0,
                        None,
                        OP.mult,
                    )
                    if g == 0:
                        # broadcasts issued after g0's slab DMAs so the
                        # gathers start as early as possible
                        nc.sync.dma_start(EDR4[:, 0:LOCAL], ea8[:])
                        nc.sync.dma_start(EDR4[:, LOCAL:], eb8[:])
                        nc.sync.dma_start(EDR[2], fr16[:])
                        for h in range(H):
                            nc.sync.dma_start(
                                EA4[:, h, :, :],
                                EDR4[2 * h : 2 * h + 1, :]
                                .partition_broadcast(128),
                            )
                            nc.sync.dma_start(
                                F1B[:, h * LOCAL : (h + 1) * LOCAL],
                                EDR[2, 2 * h : 2 * h + 1, :]
                                .partition_broadcast(128),
                            )

                # ---- Phase B: per-head all-gathers (all triggers upfront) ----
                # ---- Phase C/D: per head: stage gathered slab, then attend ----
                for h in range(H):
                    if h % 2 == 0:
                        gsrc = GGPs[l][h // 2][:].rearrange(
                            "(c p) s -> p c s", p=128
                        )
                        nc.sync.dma_start(
                            GS2[:, :, h * SLOT : (h + 2) * SLOT], gsrc
                        )
                    # f32 copy of r for the STT per-partition scalar
                    nc.vector.tensor_copy(
                        RF2[:, h, :], GS2[:, :, h * SLOT + D + 1]
                    )
                    pb = psB.tile([SLOT - 1, LOCAL], f32, tag="psB")
                    for jc2 in range(0, JC, 2):
                        p2 = p_pool.tile([128, 2 * LOCAL], bf16, tag="p2")
                        stt_mode = jc2 == 0 or (h == 0 and jc2 < 8)
                        if not stt_mode:
                            v2 = tb_pool.tile([128, 2 * LOCAL], bf16, tag="v2")
                        for k in range(2):
                            jc = jc2 + k
                            if stt_mode:
                                nc.vector.scalar_tensor_tensor(
                                    p2[:, k * LOCAL : (k + 1) * LOCAL],
                                    EA4[:, h, 1, :],
                                    RF2[:, h, jc : jc + 1],
                                    EA4[:, h, 0, :],
                                    OP.mult,
                                    OP.max,
                                )
                            else:
                                nc.scalar.activation(
                                    v2[:, k * LOCAL : (k + 1) * LOCAL],
                                    F1B[:, h * LOCAL : (h + 1) * LOCAL],
                                    AF.Exp,
                                    bias=F2GAv[:, jc, h : h + 1],
                                    scale=ALPHA,
                                )
                        if not stt_mode:
                            for k in range(2):
                                nc.vector.tensor_tensor(
                                    p2[:, k * LOCAL : (k + 1) * LOCAL],
                                    EA4[:, h, 0, :],
                                    v2[:, k * LOCAL : (k + 1) * LOCAL],
                                    OP.max,
                                )
                        pm2 = pm_pool.tile([128, 2 * LOCAL], bf16, tag="pm2")
                        nc.vector.tensor_tensor(
                            pm2[:],
                            p2[:],
                            MASK[:, jc2 * LOCAL : (jc2 + 2) * LOCAL],
                            OP.mult,
                        )
                        for k in range(2):
                            jc = jc2 + k
                            nc.tensor.matmul(
                                pb[:],
                                GS2[:, jc, h * SLOT : h * SLOT + SLOT - 1],
                                pm2[:, k * LOCAL : (k + 1) * LOCAL],
                                start=(jc == 0),
                                stop=(jc == JC - 1),
                            )

                    pending_tails.append((l, h, pb))
                    if len(pending_tails) > 1 or h == H - 1:
                        while pending_tails:
                            emit_tail(*pending_tails.pop(0))

            while pending_tails:
                emit_tail(*pending_tails.pop(0))

            # ---- final: mean over heads, ELU, write out ----
            fin = ACCb if (H - 1) % 2 == 1 else ACCa
            m1 = nm_pool.tile([D, LOCAL], f32, tag="m1")
            nc.vector.tensor_scalar(m1[:], fin[:], 1.0 / H, None, OP.mult)
            e2 = nm_pool.tile([D, LOCAL], f32, tag="e2")
            nc.scalar.activation(e2[:], m1[:], AF.Exp)
            r2 = nm_pool.tile([D, LOCAL], f32, tag="r2")
            nc.scalar.activation(r2[:], m1[:], AF.Relu)
            nc.vector.scalar_tensor_tensor(
                OUTS[:], e2[:], 1.0, r2[:], OP.min, OP.add
            )
            nc.vector.tensor_scalar(OUTS[:], OUTS[:], 1.0, None, OP.subtract)
            nc.sync.dma_start(out_d[:], OUTS[:])

    nc.compile()
    return nc


def _prep_inputs(inputs):
    x = np.asarray(inputs["x"], np.float32)
    adj = np.asarray(inputs["adj"])
    Ws = [np.asarray(inputs[k], np.float32) for k in ("W1", "W2", "W3")]
    a1s = [np.asarray(inputs[k], np.float32) for k in ("a1_1", "a1_2", "a1_3")]
    a2s = [np.asarray(inputs[k], np.float32) for k in ("a2_1", "a2_2", "a2_3")]

    wext = np.zeros((NL, F, H * SLOT), np.float32)
    for l in range(NL):
        for h in range(H):
            wext[l, :, h * SLOT : h * SLOT + D] = Ws[l][h]
            wext[l, :, h * SLOT + D] = Ws[l][h] @ a1s[l][h]
            wext[l, :, h * SLOT + D + 1] = Ws[l][h] @ a2s[l][h]
    wext_bf = np.ascontiguousarray(wext.astype(BF))
    wfa = np.zeros((NL, F, 2 * H), np.float32)
    for l in range(NL):
        for h in range(H):
            wfa[l, :, 2 * h] = Ws[l][h] @ a1s[l][h]
            wfa[l, :, 2 * h + 1] = Ws[l][h] @ a2s[l][h]
    wfa_bf = np.ascontiguousarray(wfa.astype(BF))

    mask = adj > 0
    in_maps = []
    for c in range(NCORES):
        rows = slice(c * LOCAL, (c + 1) * LOCAL)
        in_maps.append(
            {
                "xT": np.ascontiguousarray(x[rows].T).astype(BF),
                "maskT": np.ascontiguousarray(mask[rows].T).astype(BF),
                "wext": wext_bf,
                "wfa": wfa_bf,
            }
        )
    return in_maps


_CACHE = {}


def _run(inputs, trace=False):
    in_maps = _prep_inputs(inputs)
    if "nc" not in _CACHE:
        _CACHE["nc"] = build_nc()
    res = run_bass_kernel_spmd(
        _CACHE["nc"], in_maps, list(range(NCORES)), trace=trace
    )
    outs = [r["outT"] for r in res.results]
    out = np.concatenate([np.asarray(o, np.float32).T for o in outs], axis=0)
    return out, res


def kernel(**inputs) -> np.ndarray:
    out, _ = _run(inputs, trace=False)
    return out.astype(np.float32)


# revision 23
# speedup vs baseline: 1.1435x; 1.1435x over previous
"""3-layer dense GAT (N=4096, F=512, H=8 heads, D=64) on 8 TRN2 NeuronCores.

Strategy (1D row-parallel, exp(f2)-folded attention):
  - Each core owns LOCAL=512 query rows i. Per layer, each core computes its
    local hext = x_local @ [W | W@a1 | W@a2], so f1/f2 come out as extra
    matmul columns.
  - Key identity: with ec_j = exp(f2_j), r_j = exp((a-1) f2_j),
      max(exp(f1_i)exp(f2_j), exp(a f1_i)exp(a f2_j))
        = ec_j * max(exp(f1_i), exp(a f1_i) * r_j).
    So ec is folded into the stationary matrix (ec*h, and ec replaces the
    ones column so the softmax denominator Z still falls out of the same
    matmul), and the per-tile work drops to ONE scalar_tensor_tensor
    (p = max(EA, EB*r), per-partition scalar r) + ONE tensor_tensor mask
    multiply (merged over j-chunk pairs, split across Vector and GpSimd).
  - The scaling ec*h runs on the otherwise-idle Scalar engine as
    activation-Copy with a per-partition scale during PSUM extraction.
  - AllGather is issued per head (8 smaller collectives) so head 0's
    attention overlaps the remaining gathers and staging DMAs.
  - Matmul: out[d,i] += GS_h[j, 0:65].T @ pm[j,i] accumulated over 32
    j-chunks; column 64 of GS_h is ec, so PSUM row 64 is Z. h' = U/Z, ELU;
    the [d,i] orientation is the next layer's lhsT, so no transposes.
"""

import numpy as np
import ml_dtypes

import concourse.bass as bass
import concourse.mybir as mybir
from concourse import bacc, tile, masks
from concourse.bass_utils import run_bass_kernel_spmd

N = 4096
F = 512
D = 64
H = 8
NCORES = 8
LOCAL = N // NCORES          # 512 query rows per core
JC = N // 128                # 32 j-chunks
IC = LOCAL // 128            # 4 local i-chunks
FC = F // 128                # 4 contraction chunks
NL = 3
SLOT = 66                    # per-head cols: 64 ec*h + ec + r
CH = H * SLOT                # 528: per-j-chunk stride in GS
ALPHA = 0.2
f32 = mybir.dt.float32
bf16 = mybir.dt.bfloat16
BF = ml_dtypes.bfloat16
OP = mybir.AluOpType
AF = mybir.ActivationFunctionType


def build_nc():
    nc = bacc.Bacc(None, target_bir_lowering=False, num_devices=NCORES)

    xT_d = nc.dram_tensor("xT", [F, LOCAL], bf16, kind="ExternalInput")
    maskT_d = nc.dram_tensor("maskT", [N, LOCAL], bf16, kind="ExternalInput")
    wext_d = nc.dram_tensor("wext", [NL, F, H * SLOT], bf16, kind="ExternalInput")
    wfa_d = nc.dram_tensor("wfa", [NL, F, 2 * H], bf16, kind="ExternalInput")
    out_d = nc.dram_tensor("outT", [D, LOCAL], f32, kind="ExternalOutput")

    with tile.TileContext(nc) as tc:
        with (
            tc.tile_pool(name="persist", bufs=1) as pp,
            tc.tile_pool(name="ident", bufs=1) as ident_pool,
            tc.tile_pool(name="hc", bufs=4) as hc_pool,
            tc.tile_pool(name="ecd", bufs=3) as ecd_pool,
            tc.tile_pool(name="ea8", bufs=2) as ea8_pool,
            tc.tile_pool(name="ptile", bufs=6) as p_pool,
            tc.tile_pool(name="pmtile", bufs=6) as pm_pool,
            tc.tile_pool(name="norm", bufs=2) as nm_pool,
            tc.tile_pool(name="tb", bufs=6) as tb_pool,
            tc.tile_pool(name="psA", bufs=2, space="PSUM") as psA,
            tc.tile_pool(name="psB", bufs=3, space="PSUM") as psB,
            tc.tile_pool(name="psT", bufs=1, space="PSUM") as psT,
            tc.tile_pool(name="dram", bufs=1, space="DRAM") as dram,
        ):
            # ---- persistent SBUF ----
            XT = pp.tile([128, FC * LOCAL], bf16, tag="XT")        # x^T local
            MASK = pp.tile([128, JC * LOCAL], bf16, tag="MASK")    # mask^T
            WEXT = pp.tile([128, NL * FC * H * SLOT], bf16, tag="WEXT")
            GS = pp.tile([128, JC * CH], bf16, tag="GS")           # gathered stationary
            EAB = pp.tile([128, H * 2 * LOCAL], bf16, tag="EAB")   # exp(f1) bcasts
            WFA = pp.tile([128, NL * FC * 2 * H], bf16, tag="WFA")
            RF = pp.tile([128, H * JC], f32, tag="RF")             # f32 r cols (h-major)
            F1B = pp.tile([128, H * LOCAL], bf16, tag="F1B")       # raw f1 bcasts
            F2G = pp.tile([128, JC * H], f32, tag="F2G")           # gathered f2 (jc-major)
            F2GA = pp.tile([128, JC * H], f32, tag="F2GA")         # alpha * f2
            ACCa = pp.tile([D, LOCAL], f32, tag="ACCa")            # layer-3 head mean
            ACCb = pp.tile([D, LOCAL], f32, tag="ACCb")
            OUTS = pp.tile([D, LOCAL], f32, tag="OUTS")
            IDENT = ident_pool.tile([128, 128], f32)

            # DRAM bounce buffers: per-head-PAIR local slabs + gathered slabs
            LGPs = [
                dram.tile(
                    [LOCAL, 2 * SLOT], bf16, tag=f"LGP{p}", name=f"LGP{p}"
                )
                for p in range(H // 2)
            ]
            GGPs = [
                [
                    dram.tile(
                        [N, 2 * SLOT], bf16, tag=f"GGP{l}_{p}",
                        addr_space="Shared", name=f"GGP{l}_{p}",
                    )
                    for p in range(H // 2)
                ]
                for l in range(NL)
            ]
            EDR = dram.tile([3, 16, LOCAL], bf16, tag="EDR")  # exp(f1)/f1 bounce
            LG2s = [
                dram.tile([LOCAL, 4], f32, tag=f"LG2{g}", name=f"LG2{g}")
                for g in range(2)
            ]
            GG2s = [
                [
                    dram.tile(
                        [N, 4], f32, tag=f"GG2_{l}_{g}", addr_space="Shared",
                        name=f"GG2_{l}_{g}",
                    )
                    for g in range(2)
                ]
                for l in range(NL)
            ]

            # views
            X2 = XT[:].rearrange("p (fc i) -> p fc i", i=LOCAL)
            M2 = MASK[:].rearrange("p (c i) -> p c i", i=LOCAL)
            W4 = WEXT[:].rearrange("p (l fc s) -> p l fc s", l=NL, fc=FC)
            WFA4 = WFA[:].rearrange("p (l fc s) -> p l fc s", l=NL, fc=FC)
            GS2 = GS[:].rearrange("p (c s) -> p c s", s=CH)
            RF2 = RF[:].rearrange("p (h c) -> p h c", h=H)
            F2Gv = F2G[:].rearrange("p (c s) -> p c s", s=H)
            F2GAv = F2GA[:].rearrange("p (c s) -> p c s", s=H)
            EA4 = EAB[:].rearrange("p (h t i) -> p h t i", h=H, t=2)

            # ---- one-time loads ----
            nc.sync.dma_start(X2, xT_d[:].rearrange("(fc p) i -> p fc i", p=128))
            nc.sync.dma_start(M2, maskT_d[:].rearrange("(c p) i -> p c i", p=128))
            nc.sync.dma_start(
                W4, wext_d[:].rearrange("l (fc p) s -> p l fc s", p=128)
            )
            nc.sync.dma_start(
                WFA4, wfa_d[:].rearrange("l (fc p) s -> p l fc s", p=128)
            )
            masks.make_identity(nc, IDENT[:])
            NEG1 = pp.tile([128, 1], f32, tag="NEG1")
            nc.gpsimd.memset(NEG1[:], -1.0)


            def emit_tail(l, h, pb):
                # normalize by Z (PSUM row 64) and apply ELU
                # 1/Z: DMA-reshape Z [1,512] -> [128,4] so the iterative
                # divide runs 4-deep across 128 lanes, then reshape back.
                zrow1 = nm_pool.tile([1, LOCAL], f32, tag="zrow1")
                nc.scalar.copy(zrow1[:], pb[D : D + 1, :])
                zz4 = nm_pool.tile([128, LOCAL // 128], f32, tag="zz4")
                nc.sync.dma_start(zz4[:], zrow1[:])
                rz4 = nm_pool.tile([128, LOCAL // 128], f32, tag="rz4")
                nc.vector.reciprocal(rz4[:], zz4[:])
                r1 = nm_pool.tile([1, LOCAL], f32, tag="r1")
                nc.sync.dma_start(r1[:], rz4[:])
                rb2 = nm_pool.tile([D, LOCAL], f32, tag="rb2")
                nc.gpsimd.partition_broadcast(rb2[:], r1[:], channels=D)
                y = nm_pool.tile([D, LOCAL], f32, tag="y")
                nc.vector.tensor_tensor(y[:], pb[0:D, :], rb2[:], OP.mult)
                ee = nm_pool.tile([D, LOCAL], f32, tag="ee")
                nc.scalar.activation(ee[:], y[:], AF.Exp)
                ry = nm_pool.tile([D, LOCAL], f32, tag="ry")
                nc.scalar.activation(ry[:], y[:], AF.Relu)
                z1 = nm_pool.tile([D, LOCAL], f32, tag="z1")
                nc.vector.scalar_tensor_tensor(
                    z1[:], ee[:], 1.0, ry[:], OP.min, OP.add
                )
                if l < NL - 1:
                    poff = (h % 2) * D
                    dst = X2[poff : poff + D, h // 2, :]
                    nc.scalar.activation(dst, z1[:], AF.Identity, bias=NEG1[0:D, :])
                else:
                    ey = nm_pool.tile([D, LOCAL], f32, tag="ey")
                    nc.scalar.activation(ey[:], z1[:], AF.Identity, bias=NEG1[0:D, :])
                    if h == 0:
                        nc.vector.tensor_copy(ACCa[:], ey[:])
                    else:
                        acc_src, dst_acc = (
                            (ACCa, ACCb) if h % 2 == 1 else (ACCb, ACCa)
                        )
                        nc.vector.tensor_tensor(
                            dst_acc[:], acc_src[:], ey[:], OP.add
                        )

            pending_tails = []

            for l in range(NL):
                while pending_tails:
                    emit_tail(*pending_tails.pop(0))
                # ---- Phase A2 (first): f1/f2 rows via matmul, exp, bcast ----
                psf = psT.tile([2 * H, LOCAL], f32, tag="psf")
                for fc in range(FC):
                    nc.tensor.matmul(
                        psf[:],
                        WFA4[:, l, fc, :],
                        X2[:, fc, :],
                        start=(fc == 0),
                        stop=(fc == FC - 1),
                    )
                ea8 = ea8_pool.tile([16, LOCAL], bf16, tag="ea8")
                eb8 = ea8_pool.tile([16, LOCAL], bf16, tag="eb8")
                nc.scalar.activation(ea8[:], psf[:], AF.Exp)
                nc.scalar.activation(eb8[:], psf[:], AF.Exp, scale=ALPHA)
                fr16 = ea8_pool.tile([16, LOCAL], bf16, tag="fr16")
                nc.vector.tensor_copy(fr16[:], psf[:])

                # ---- Phase A: local hext = x_local @ Wext, 4 heads/group ----
                for g in range(2):
                    for ic in range(IC):
                        ps = psA.tile([128, 4 * SLOT], f32, tag="psA")
                        for fc in range(FC):
                            nc.tensor.matmul(
                                ps[:],
                                X2[:, fc, ic * 128 : (ic + 1) * 128],
                                W4[:, l, fc, g * 4 * SLOT : (g + 1) * 4 * SLOT],
                                start=(fc == 0),
                                stop=(fc == FC - 1),
                            )
                        psv = ps[:].rearrange("p (h s) -> p h s", s=SLOT)
                        rows = slice(ic * 128, (ic + 1) * 128)
                        sg4 = hc_pool.tile([128, 4 * SLOT], bf16, tag="sg4")
                        sg4v = sg4[:].rearrange("p (h s) -> p h s", s=SLOT)
                        # folded rows: [ec*h | ec | r]; f2 staged (f32)
                        # separately as the activation-branch bias
                        ecf = ecd_pool.tile([128, 4], f32, tag="ecf")
                        nc.scalar.activation(ecf[:], psv[:, :, D + 1], AF.Exp)
                        rf = ecd_pool.tile([128, 4], f32, tag="rf")
                        nc.scalar.activation(
                            rf[:], psv[:, :, D + 1], AF.Exp, scale=ALPHA - 1.0
                        )
                        for hh in range(4):
                            nc.scalar.activation(
                                sg4v[:, hh, 0:D],
                                psv[:, hh, 0:D],
                                AF.Copy,
                                scale=ecf[:, hh : hh + 1],
                            )
                        nc.vector.tensor_copy(sg4v[:, :, D], ecf[:])
                        nc.vector.tensor_copy(sg4v[:, :, D + 1], rf[:])
                        f2c = ecd_pool.tile([128, 4], f32, tag="f2c")
                        nc.scalar.copy(f2c[:], psv[:, :, D + 1])
                        nc.sync.dma_start(LG2s[g][rows, :], f2c[:])
                        for hh in range(4):
                            h = g * 4 + hh
                            nc.sync.dma_start(
                                LGPs[h // 2][
                                    rows, (h % 2) * SLOT : (h % 2 + 1) * SLOT
                                ],
                                sg4v[:, hh, :],
                            )
                    # this head-group's slabs are complete: gather them now
                    nc.gpsimd.collective_compute(
                        "AllGather",
                        OP.bypass,
                        replica_groups=[list(range(NCORES))],
                        ins=[LG2s[g].opt()],
                        outs=[GG2s[l][g].opt()],
                    )
                    for p in (2 * g, 2 * g + 1):
                        nc.gpsimd.collective_compute(
                            "AllGather",
                            OP.bypass,
                            replica_groups=[list(range(NCORES))],
                            ins=[LGPs[p].opt()],
                            outs=[GGPs[l][p].opt()],
                        )
                    nc.sync.dma_start(
                        F2Gv[:, :, g * 4 : (g + 1) * 4],
                        GG2s[l][g][:].rearrange("(c p) s -> p c s", p=128),
                    )
                    nc.vector.tensor_scalar(
                        F2GAv[:, :, g * 4 : (g + 1) * 4],
                        F2Gv[:, :, g * 4 : (g + 1) * 4],
                        ALPHA - 1.0,
                        None,
                        OP.mult,
                    )

                nc.sync.dma_start(EDR[0], ea8[:])
                nc.sync.dma_start(EDR[1], eb8[:])
                nc.sync.dma_start(EDR[2], fr16[:])
                for h in range(H):
                    for t in range(2):
                        nc.sync.dma_start(
                            EA4[:, h, t, :],
                            EDR[t, 2 * h : 2 * h + 1, :].partition_broadcast(128),
                        )
                    nc.sync.dma_start(
                        F1B[:, h * LOCAL : (h + 1) * LOCAL],
                        EDR[2, 2 * h : 2 * h + 1, :].partition_broadcast(128),
                    )

                # ---- Phase B: per-head all-gathers (all triggers upfront) ----
                # ---- Phase C/D: per head: stage gathered slab, then attend ----
                for h in range(H):
                    if h % 2 == 0:
                        gsrc = GGPs[l][h // 2][:].rearrange(
                            "(c p) s -> p c s", p=128
                        )
                        nc.sync.dma_start(
                            GS2[:, :, h * SLOT : (h + 2) * SLOT], gsrc
                        )
                    # f32 copy of r for the STT per-partition scalar
                    nc.vector.tensor_copy(
                        RF2[:, h, :], GS2[:, :, h * SLOT + D + 1]
                    )
                    pb = psB.tile([SLOT - 1, LOCAL], f32, tag="psB")
                    for jc2 in range(0, JC, 2):
                        p2 = p_pool.tile([128, 2 * LOCAL], bf16, tag="p2")
                        if jc2 > 0:
                            v2 = tb_pool.tile([128, 2 * LOCAL], bf16, tag="v2")
                        for k in range(2):
                            jc = jc2 + k
                            if jc2 == 0:
                                nc.vector.scalar_tensor_tensor(
                                    p2[:, k * LOCAL : (k + 1) * LOCAL],
                                    EA4[:, h, 1, :],
                                    RF2[:, h, jc : jc + 1],
                                    EA4[:, h, 0, :],
                                    OP.mult,
                                    OP.max,
                                )
                            else:
                                nc.scalar.activation(
                                    v2[:, k * LOCAL : (k + 1) * LOCAL],
                                    F1B[:, h * LOCAL : (h + 1) * LOCAL],
                                    AF.Exp,
                                    bias=F2GAv[:, jc, h : h + 1],
                                    scale=ALPHA,
                                )
                        if jc2 > 0:
                            for k in range(2):
                                nc.vector.tensor_tensor(
                                    p2[:, k * LOCAL : (k + 1) * LOCAL],
                                    EA4[:, h, 0, :],
                                    v2[:, k * LOCAL : (k + 1) * LOCAL],
                                    OP.max,
                                )
                        pm2 = pm_pool.tile([128, 2 * LOCAL], bf16, tag="pm2")
                        nc.vector.tensor_tensor(
                            pm2[:],
                            p2[:],
                            MASK[:, jc2 * LOCAL : (jc2 + 2) * LOCAL],
                            OP.mult,
                        )
                        for k in range(2):
                            jc = jc2 + k
                            nc.tensor.matmul(
                                pb[:],
                                GS2[:, jc, h * SLOT : h * SLOT + SLOT - 1],
                                pm2[:, k * LOCAL : (k + 1) * LOCAL],
                                start=(jc == 0),
                                stop=(jc == JC - 1),
                            )

                    pending_tails.append((l, h, pb))
                    if len(pending_tails) > 1 or h == H - 1:
                        while pending_tails:
                            emit_tail(*pending_tails.pop(0))

            while pending_tails:
                emit_tail(*pending_tails.pop(0))

            # ---- final: mean over heads, ELU, write out ----
            fin = ACCb if (H - 1) % 2 == 1 else ACCa
            m1 = nm_pool.tile([D, LOCAL], f32, tag="m1")
            nc.vector.tensor_scalar(m1[:], fin[:], 1.0 / H, None, OP.mult)
            e2 = nm_pool.tile([D, LOCAL], f32, tag="e2")
            nc.scalar.activation(e2[:], m1[:], AF.Exp)
            r2 = nm_pool.tile([D, LOCAL], f32, tag="r2")
            nc.scalar.activation(r2[:], m1[:], AF.Relu)
            nc.vector.scalar_tensor_tensor(
                OUTS[:], e2[:], 1.0, r2[:], OP.min, OP.add
            )
            nc.vector.tensor_scalar(OUTS[:], OUTS[:], 1.0, None, OP.subtract)
            nc.sync.dma_start(out_d[:], OUTS[:])

    nc.compile()
    return nc


def _prep_inputs(inputs):
    x = np.asarray(inputs["x"], np.float32)
    adj = np.asarray(inputs["adj"])
    Ws = [np.asarray(inputs[k], np.float32) for k in ("W1", "W2", "W3")]
    a1s = [np.asarray(inputs[k], np.float32) for k in ("a1_1", "a1_2", "a1_3")]
    a2s = [np.asarray(inputs[k], np.float32) for k in ("a2_1", "a2_2", "a2_3")]

    wext = np.zeros((NL, F, H * SLOT), np.float32)
    for l in range(NL):
        for h in range(H):
            wext[l, :, h * SLOT : h * SLOT + D] = Ws[l][h]
            wext[l, :, h * SLOT + D] = Ws[l][h] @ a1s[l][h]
            wext[l, :, h * SLOT + D + 1] = Ws[l][h] @ a2s[l][h]
    wext_bf = np.ascontiguousarray(wext.astype(BF))
    wfa = np.zeros((NL, F, 2 * H), np.float32)
    for l in range(NL):
        for h in range(H):
            wfa[l, :, 2 * h] = Ws[l][h] @ a1s[l][h]
            wfa[l, :, 2 * h + 1] = Ws[l][h] @ a2s[l][h]
    wfa_bf = np.ascontiguousarray(wfa.astype(BF))

    mask = adj > 0
    in_maps = []
    for c in range(NCORES):
        rows = slice(c * LOCAL, (c + 1) * LOCAL)
        in_maps.append(
            {
                "xT": np.ascontiguousarray(x[rows].T).astype(BF),
                "maskT": np.ascontiguousarray(mask[rows].T).astype(BF),
                "wext": wext_bf,
                "wfa": wfa_bf,
            }
        )
    return in_maps


_CACHE = {}


def _run(inputs, trace=False):
    in_maps = _prep_inputs(inputs)
    if "nc" not in _CACHE:
        _CACHE["nc"] = build_nc()
    res = run_bass_kernel_spmd(
        _CACHE["nc"], in_maps, list(range(NCORES)), trace=trace
    )
    outs = [r["outT"] for r in res.results]
    out = np.concatenate([np.asarray(o, np.float32).T for o in outs], axis=0)
    return out, res


def kernel(**inputs) -> np.ndarray:
    out, _ = _run(inputs, trace=False)
    return out.astype(np.float32)
